# revision 11
# baseline (speedup 1.0000x reference)
"""CLIP loss kernel for trn2, 8 NeuronCores, data-parallel over the batch dim.

Strategy (per core c of 8, SPMD):
  inputs: img slice [1024, 512] f32, spec slice [1024, 512] f32 (rows
  1024c..1024c+1023 of each modality).
  1. sumsq of both slices (DVE tensor_tensor_reduce) -> 1/norms via
     exp(-0.5*ln(max(ss, eps^2))) on ACT (Ln+Exp share one table set).
  2. spec slice normalized (per-partition scalar mul, bf16 out), transposed
     via PE into [512, 1024] bf16, AllGather -> full [512, 8192] spec^T.
     img slice kept RAW (norm folded into the exp scale later), transposed
     via PE into [512, 1024] bf16.
  3. logits block: out[m=img rows, n=spec cols] = imgT.T @ specT, bf16,
     PSUM f32, tiles [128, 2048].
  4. ACT Exp with scale = logit_scale * (1/|img_row|) per partition;
     accum_out gives row-sums of exp for free. exp tile (bf16, SBUF)
     accumulated into racc[128, 8192] (DVE add) = column partial sums
     stratified by partition; final 128-partition reduce via PE ones-matmul.
  5. diag: raw img.spec dot per row (DVE), combined with norms on host.
Host: gathers per-core row-sums / column partials / diag pieces, takes logs
and means (O(N) numpy) -> scalar loss.
"""

import os
from contextlib import ExitStack

import numpy as np

import concourse.bass as bass
import concourse.mybir as mybir
from concourse import bacc, tile
from concourse.bass_utils import run_bass_kernel_spmd
from concourse.masks import make_identity

N, D, C = 8192, 512, 8
NL = N // C  # 1024 local rows per core
P = 128
T = NL // P  # 8 natural [128, 512] tiles per modality slice
KC = D // P  # 4 contraction chunks
G = 4        # column groups per core block
GW = N // G  # 2048 columns per group

f32 = mybir.dt.float32
bf16 = mybir.dt.bfloat16
FA = mybir.ActivationFunctionType
ALU = mybir.AluOpType

_cache: dict = {}


def _build(scale: float, use_cc: bool = True):
    nc = bacc.Bacc("TRN2", target_bir_lowering=False, debug=False, num_devices=C)
    img = nc.dram_tensor("img", [NL, D], f32, kind="ExternalInput")
    spec = nc.dram_tensor("spec", [NL, D], f32, kind="ExternalInput")
    rowsum_o = nc.dram_tensor("rowsum", [P, T], f32, kind="ExternalOutput")
    colsum_o = nc.dram_tensor("colsum", [1, N], f32, kind="ExternalOutput")
    dotd_o = nc.dram_tensor("dotd", [P, T], f32, kind="ExternalOutput")
    rni_o = nc.dram_tensor("rni", [P, T], f32, kind="ExternalOutput")
    rns_o = nc.dram_tensor("rns", [P, T], f32, kind="ExternalOutput")

    with tile.TileContext(nc) as tc, ExitStack() as ctx:
        const = ctx.enter_context(tc.tile_pool(name="const", bufs=1))
        natp = ctx.enter_context(tc.tile_pool(name="nat", bufs=T))
        scp = ctx.enter_context(tc.tile_pool(name="scr", bufs=2))
        spn = ctx.enter_context(tc.tile_pool(name="specn", bufs=4))
        pers = ctx.enter_context(tc.tile_pool(name="pers", bufs=1))
        ps = ctx.enter_context(tc.tile_pool(name="ps", bufs=2, space="PSUM"))
        ep = ctx.enter_context(tc.tile_pool(name="e", bufs=3))
        dramp = ctx.enter_context(tc.tile_pool(name="dram", bufs=1, space="DRAM"))

        ident_f = const.tile([P, P], f32, name="identf")
        make_identity(nc, ident_f)
        ident_b = const.tile([P, P], bf16, name="identb")
        nc.vector.tensor_copy(ident_b, ident_f)
        ones_b = const.tile([P, 1], bf16, name="onesb")
        nc.vector.memset(ones_b, 1.0)

        imgT = [pers.tile([P, NL], bf16, name=f"imgT{k}") for k in range(KC)]
        specT = [pers.tile([P, N], bf16, name=f"specT{k}") for k in range(KC)]
        stage = [pers.tile([P, NL], bf16, name=f"stage{k}") for k in range(KC)]
        racc = pers.tile([P, N], bf16, name="racc")
        rowacc = pers.tile([P, T, G], f32, name="rowacc")
        ssi = pers.tile([P, T], f32, name="ssi")
        sss = pers.tile([P, T], f32, name="sss")
        rni = pers.tile([P, T], f32, name="rni")
        rns = pers.tile([P, T], f32, name="rns")
        sci = pers.tile([P, T], f32, name="sci")
        dotd = pers.tile([P, T], f32, name="dotd")
        lntmp = pers.tile([P, T], f32, name="lntmp")
        lntmp2 = pers.tile([P, T], f32, name="lntmp2")
        rows = pers.tile([P, T], f32, name="rows")
        colsb = pers.tile([1, N], f32, name="colsb")

        cc_in = dramp.tile([D, NL], bf16, name="cc_in")
        cc_out = dramp.tile([C * D, NL], bf16, addr_space="Shared", name="cc_out")

        # ---- load natural tiles ----
        img_nat, spec_nat = [], []
        for t in range(T):
            st = natp.tile([P, D], f32, tag="specnat")
            nc.sync.dma_start(st, spec.ap()[t * P : (t + 1) * P, :])
            spec_nat.append(st)
        for t in range(T):
            it = natp.tile([P, D], f32, tag="imgnat")
            nc.sync.dma_start(it, img.ap()[t * P : (t + 1) * P, :])
            img_nat.append(it)

        # ---- spec norms first (the collective is on the critical path) ----
        for t in range(T):
            s2 = scp.tile([P, D], f32, tag="scr")
            nc.scalar.activation(
                s2, spec_nat[t], FA.Square, accum_out=sss[:, t : t + 1]
            )
        nc.vector.tensor_scalar_max(sss, sss, 1.0e-6)
        nc.scalar.activation(lntmp, sss, FA.Ln)
        nc.scalar.activation(rns, lntmp, FA.Exp, scale=-0.5)

        # ---- normalize spec (bf16) + PE transpose + stage + AllGather ----
        for th in range(2):
            specn = []
            for tt in range(4):
                t = 4 * th + tt
                sn = spn.tile([P, D], bf16, tag="specn")
                nc.vector.tensor_scalar_mul(sn, spec_nat[t], rns[:, t : t + 1])
                specn.append(sn)
            for k in range(KC):
                pt = ps.tile([P, 512], bf16, tag="mm")
                for tt in range(4):
                    nc.tensor.transpose(
                        pt[:, 128 * tt : 128 * (tt + 1)],
                        specn[tt][:, 128 * k : 128 * (k + 1)],
                        ident_b,
                    )
                nc.vector.tensor_copy(stage[k][:, 512 * th : 512 * (th + 1)], pt)
        for k in range(KC):
            nc.sync.dma_start(cc_in[128 * k : 128 * (k + 1), :], stage[k])
        if use_cc:
            nc.gpsimd.collective_compute(
                "AllGather",
                ALU.bypass,
                replica_groups=[list(range(C))],
                ins=[cc_in.opt()],
                outs=[cc_out.opt()],
            )

        # ---- img norms, diag dots, img transpose (overlap the collective) ----
        for t in range(T):
            s1 = scp.tile([P, D], f32, tag="scr")
            nc.scalar.activation(
                s1, img_nat[t], FA.Square, accum_out=ssi[:, t : t + 1]
            )
            s3 = scp.tile([P, D], f32, tag="scr")
            nc.vector.tensor_mul(out=s3, in0=img_nat[t], in1=spec_nat[t])
            nc.vector.reduce_sum(
                dotd[:, t : t + 1], s3, axis=mybir.AxisListType.X
            )
        nc.vector.tensor_scalar_max(ssi, ssi, 1.0e-6)
        nc.scalar.activation(lntmp2, ssi, FA.Ln)
        nc.scalar.activation(rni, lntmp2, FA.Exp, scale=-0.5)
        nc.vector.tensor_scalar_mul(sci, rni, scale)

        for th in range(2):
            for k in range(KC):
                pt = ps.tile([P, 512], f32, tag="mm")
                for tt in range(4):
                    t = 4 * th + tt
                    nc.tensor.transpose(
                        pt[:, 128 * tt : 128 * (tt + 1)],
                        img_nat[t][:, 128 * k : 128 * (k + 1)],
                        ident_f,
                    )
                nc.vector.tensor_copy(imgT[k][:, 512 * th : 512 * (th + 1)], pt)

        # ---- load gathered spec^T: core chunk c -> columns 1024c.. ----
        for c in range(C):
            for k in range(KC):
                if use_cc:
                    src = cc_out[D * c + 128 * k : D * c + 128 * (k + 1), :]
                else:  # debug: replicate the local slice (numerically wrong)
                    src = cc_in[128 * k : 128 * (k + 1), :]
                nc.sync.dma_start(specT[k][:, NL * c : NL * (c + 1)], src)

        # ---- main loop: logits block, exp, row/col accumulation ----
        with nc.allow_low_precision("bf16 exp-sum accumulation, error ~0.5% -> <1e-3 on loss"):
            for m in range(T):
                for g in range(G):
                    pm = ps.tile([P, GW], f32, tag="mm")
                    for ns in range(GW // 512):
                        for k in range(KC):
                            nc.tensor.matmul(
                                pm[:, 512 * ns : 512 * (ns + 1)],
                                imgT[k][:, P * m : P * (m + 1)],
                                specT[k][:, GW * g + 512 * ns : GW * g + 512 * (ns + 1)],
                                start=(k == 0),
                                stop=(k == KC - 1),
                            )
                    e = ep.tile([P, GW], bf16, tag="e")
                    nc.scalar.activation(
                        e, pm, FA.Exp,
                        scale=sci[:, m : m + 1],
                        accum_out=rowacc[:, m, g : g + 1],
                    )
                    gsl = racc[:, GW * g : GW * (g + 1)]
                    if m == 0:
                        nc.vector.tensor_copy(gsl, e)
                    else:
                        nc.vector.tensor_add(out=gsl, in0=gsl, in1=e)

        # ---- tails ----
        nc.vector.reduce_sum(rows, rowacc[:, :, :], axis=mybir.AxisListType.X)
        nc.sync.dma_start(rowsum_o.ap(), rows)
        for j in range(N // 512):
            pc = ps.tile([1, 512], f32, tag="mm")
            nc.tensor.matmul(
                pc, ones_b, racc[:, 512 * j : 512 * (j + 1)], start=True, stop=True
            )
            eng = nc.vector if j % 2 == 0 else nc.scalar
            if eng is nc.vector:
                nc.vector.tensor_copy(colsb[0:1, 512 * j : 512 * (j + 1)], pc)
            else:
                nc.scalar.activation(
                    colsb[0:1, 512 * j : 512 * (j + 1)], pc, FA.Copy
                )
        nc.sync.dma_start(colsum_o.ap(), colsb)
        nc.sync.dma_start(dotd_o.ap(), dotd)
        nc.sync.dma_start(rni_o.ap(), rni)
        nc.sync.dma_start(rns_o.ap(), rns)

    nc.compile()
    return nc


def _ensure_ntff_hook():
    """antenv.axon_hooks is absent on this image; provide the tiny get/set
    registry and register trn_agent_boot's ctypes NTFF hook so trace=True
    works. Only used from test runs (KERNEL_TRACE=1)."""
    import sys
    import types

    try:
        import antenv.axon_hooks  # noqa: F401
        return
    except ImportError:
        pass
    mod = types.ModuleType("antenv.axon_hooks")
    _state = {"hook": None}
    mod.set_axon_ntff_profile_hook = lambda h: _state.__setitem__("hook", h)
    mod.get_axon_ntff_profile_hook = lambda: _state["hook"]
    import antenv

    sys.modules["antenv.axon_hooks"] = mod
    antenv.axon_hooks = mod
    try:
        from trn_agent_boot.trn_boot import _ntff_profile_via_ctypes

        mod.set_axon_ntff_profile_hook(
            _ntff_profile_via_ctypes("/opt/axon/libaxon_pjrt.so")
        )
    except Exception as e:  # degrade to no tracing
        print(f"NTFF hook setup failed: {e}")


def kernel(image_features, spectrum_features, logit_scale):
    scale = float(np.asarray(logit_scale))
    key = round(scale, 9)
    if key not in _cache:
        _cache[key] = _build(scale)
    nc = _cache[key]

    img = np.ascontiguousarray(np.asarray(image_features, dtype=np.float32))
    spec = np.ascontiguousarray(np.asarray(spectrum_features, dtype=np.float32))
    in_maps = [
        {"img": img[c * NL : (c + 1) * NL], "spec": spec[c * NL : (c + 1) * NL]}
        for c in range(C)
    ]
    trace = os.environ.get("KERNEL_TRACE") == "1"
    if trace:
        _ensure_ntff_hook()
    res = run_bass_kernel_spmd(nc, in_maps, core_ids=list(range(C)), trace=trace)
    if trace:
        print(f"HW exec time: {res.exec_time_ns} ns (mean {res.mean_exec_time_ns})")

    rs = np.stack([r["rowsum"] for r in res.results]).astype(np.float64)   # [C,P,T]
    cs = np.stack([r["colsum"][0] for r in res.results]).astype(np.float64)  # [C,N]
    dd = np.stack([r["dotd"] for r in res.results]).astype(np.float64)
    ri = np.stack([r["rni"] for r in res.results]).astype(np.float64)
    rr = np.stack([r["rns"] for r in res.results]).astype(np.float64)

    diag_sum = float(np.sum(scale * dd * ri * rr))
    lse_i_sum = float(np.sum(np.log(rs)))
    col_total = cs.sum(axis=0)
    lse_s_sum = float(np.sum(np.log(col_total)))
    loss = 0.5 * ((lse_i_sum - diag_sum) / N + (lse_s_sum - diag_sum) / N)
    return np.float32(loss)


# revision 17
# speedup vs baseline: 1.0131x; 1.0131x over previous
"""CLIP loss kernel for trn2, 8 NeuronCores, data-parallel over the batch dim.

Strategy (per core c of 8, SPMD):
  inputs: img slice [1024, 512] f32, spec slice [1024, 512] f32 (rows
  1024c..1024c+1023 of each modality).
  1. sumsq of both slices (DVE tensor_tensor_reduce) -> 1/norms via
     exp(-0.5*ln(max(ss, eps^2))) on ACT (Ln+Exp share one table set).
  2. spec slice normalized (per-partition scalar mul, bf16 out), transposed
     via PE into [512, 1024] bf16, AllGather -> full [512, 8192] spec^T.
     img slice kept RAW (norm folded into the exp scale later), transposed
     via PE into [512, 1024] bf16.
  3. logits block: out[m=img rows, n=spec cols] = imgT.T @ specT, bf16,
     PSUM f32, tiles [128, 2048].
  4. ACT Exp with scale = logit_scale * (1/|img_row|) per partition;
     accum_out gives row-sums of exp for free. exp tile (bf16, SBUF)
     accumulated into racc[128, 8192] (DVE add) = column partial sums
     stratified by partition; final 128-partition reduce via PE ones-matmul.
  5. diag: raw img.spec dot per row (DVE), combined with norms on host.
Host: gathers per-core row-sums / column partials / diag pieces, takes logs
and means (O(N) numpy) -> scalar loss.
"""

import os
from contextlib import ExitStack

import numpy as np

import concourse.bass as bass
import concourse.mybir as mybir
from concourse import bacc, tile
from concourse.bass_utils import run_bass_kernel_spmd
from concourse.masks import make_identity

N, D, C = 8192, 512, 8
NL = N // C  # 1024 local rows per core
P = 128
T = NL // P  # 8 natural [128, 512] tiles per modality slice
KC = D // P  # 4 contraction chunks
G = 4        # column groups per core block
GW = N // G  # 2048 columns per group

f32 = mybir.dt.float32
bf16 = mybir.dt.bfloat16
FA = mybir.ActivationFunctionType
ALU = mybir.AluOpType

_cache: dict = {}


def _build(scale: float, use_cc: bool = True):
    nc = bacc.Bacc("TRN2", target_bir_lowering=False, debug=False, num_devices=C)
    img = nc.dram_tensor("img", [NL, D], f32, kind="ExternalInput")
    spec = nc.dram_tensor("spec", [NL, D], f32, kind="ExternalInput")
    rowsum_o = nc.dram_tensor("rowsum", [P, T], f32, kind="ExternalOutput")
    racc_o = nc.dram_tensor("racc_o", [P, N], bf16, kind="ExternalOutput")
    dotd_o = nc.dram_tensor("dotd", [P, T], f32, kind="ExternalOutput")
    rni_o = nc.dram_tensor("rni", [P, T], f32, kind="ExternalOutput")
    rns_o = nc.dram_tensor("rns", [P, T], f32, kind="ExternalOutput")

    with tile.TileContext(nc) as tc, ExitStack() as ctx:
        const = ctx.enter_context(tc.tile_pool(name="const", bufs=1))
        natp = ctx.enter_context(tc.tile_pool(name="nat", bufs=T))
        scp = ctx.enter_context(tc.tile_pool(name="scr", bufs=2))
        spn = ctx.enter_context(tc.tile_pool(name="specn", bufs=4))
        pers = ctx.enter_context(tc.tile_pool(name="pers", bufs=1))
        ps = ctx.enter_context(tc.tile_pool(name="ps", bufs=2, space="PSUM"))
        ep = ctx.enter_context(tc.tile_pool(name="e", bufs=3))
        dramp = ctx.enter_context(tc.tile_pool(name="dram", bufs=1, space="DRAM"))

        ident_f = const.tile([P, P], f32, name="identf")
        make_identity(nc, ident_f)
        ident_b = const.tile([P, P], bf16, name="identb")
        nc.vector.tensor_copy(ident_b, ident_f)

        imgT = [pers.tile([P, NL], bf16, name=f"imgT{k}") for k in range(KC)]
        specT = [pers.tile([P, N], bf16, name=f"specT{k}") for k in range(KC)]
        stage = [pers.tile([P, NL], bf16, name=f"stage{k}") for k in range(KC)]
        racc = pers.tile([P, N], bf16, name="racc")
        rowacc = pers.tile([P, T, G], f32, name="rowacc")
        ssi = pers.tile([P, T], f32, name="ssi")
        sss = pers.tile([P, T], f32, name="sss")
        rni = pers.tile([P, T], f32, name="rni")
        rns = pers.tile([P, T], f32, name="rns")
        sci = pers.tile([P, T], f32, name="sci")
        dotd = pers.tile([P, T], f32, name="dotd")
        lntmp = pers.tile([P, T], f32, name="lntmp")
        lntmp2 = pers.tile([P, T], f32, name="lntmp2")
        rows = pers.tile([P, T], f32, name="rows")

        cc_in = dramp.tile([D, NL], bf16, name="cc_in")
        cc_out = dramp.tile([C * D, NL], bf16, addr_space="Shared", name="cc_out")

        # ---- load natural tiles ----
        img_nat, spec_nat = [], []
        for t in range(T):
            st = natp.tile([P, D], f32, tag="specnat")
            nc.sync.dma_start(st, spec.ap()[t * P : (t + 1) * P, :])
            spec_nat.append(st)
        for t in range(T):
            it = natp.tile([P, D], f32, tag="imgnat")
            nc.sync.dma_start(it, img.ap()[t * P : (t + 1) * P, :])
            img_nat.append(it)

        # ---- spec norms first (the collective is on the critical path) ----
        for t in range(T):
            s2 = scp.tile([P, D], f32, tag="scr")
            nc.scalar.activation(
                s2, spec_nat[t], FA.Square, accum_out=sss[:, t : t + 1]
            )
        nc.vector.tensor_scalar_max(sss, sss, 1.0e-6)
        nc.scalar.activation(lntmp, sss, FA.Ln)
        nc.scalar.activation(rns, lntmp, FA.Exp, scale=-0.5)

        # ---- normalize spec (bf16) + PE transpose + stage + AllGather ----
        for th in range(2):
            specn = []
            for tt in range(4):
                t = 4 * th + tt
                sn = spn.tile([P, D], bf16, tag="specn")
                nc.vector.tensor_scalar_mul(sn, spec_nat[t], rns[:, t : t + 1])
                specn.append(sn)
            for k in range(KC):
                pt = ps.tile([P, 512], bf16, tag="mm")
                for tt in range(4):
                    nc.tensor.transpose(
                        pt[:, 128 * tt : 128 * (tt + 1)],
                        specn[tt][:, 128 * k : 128 * (k + 1)],
                        ident_b,
                    )
                nc.vector.tensor_copy(stage[k][:, 512 * th : 512 * (th + 1)], pt)
        for k in range(KC):
            nc.sync.dma_start(cc_in[128 * k : 128 * (k + 1), :], stage[k])
        if use_cc:
            nc.gpsimd.collective_compute(
                "AllGather",
                ALU.bypass,
                replica_groups=[list(range(C))],
                ins=[cc_in.opt()],
                outs=[cc_out.opt()],
            )

        # ---- img norms, diag dots, img transpose (overlap the collective) ----
        for t in range(T):
            s1 = scp.tile([P, D], f32, tag="scr")
            nc.scalar.activation(
                s1, img_nat[t], FA.Square, accum_out=ssi[:, t : t + 1]
            )
            s3 = scp.tile([P, D], f32, tag="scr")
            nc.vector.tensor_mul(out=s3, in0=img_nat[t], in1=spec_nat[t])
            nc.vector.reduce_sum(
                dotd[:, t : t + 1], s3, axis=mybir.AxisListType.X
            )
        nc.vector.tensor_scalar_max(ssi, ssi, 1.0e-6)
        nc.scalar.activation(lntmp2, ssi, FA.Ln)
        nc.scalar.activation(rni, lntmp2, FA.Exp, scale=-0.5)
        nc.vector.tensor_scalar_mul(sci, rni, scale)

        for th in range(2):
            for k in range(KC):
                pt = ps.tile([P, 512], f32, tag="mm")
                for tt in range(4):
                    t = 4 * th + tt
                    nc.tensor.transpose(
                        pt[:, 128 * tt : 128 * (tt + 1)],
                        img_nat[t][:, 128 * k : 128 * (k + 1)],
                        ident_f,
                    )
                nc.vector.tensor_copy(imgT[k][:, 512 * th : 512 * (th + 1)], pt)

        # ---- load gathered spec^T: core chunk c -> columns 1024c.. ----
        for c in range(C):
            for k in range(KC):
                if use_cc:
                    src = cc_out[D * c + 128 * k : D * c + 128 * (k + 1), :]
                else:  # debug: replicate the local slice (numerically wrong)
                    src = cc_in[128 * k : 128 * (k + 1), :]
                nc.sync.dma_start(specT[k][:, NL * c : NL * (c + 1)], src)

        # ---- main loop: logits block, exp, row/col accumulation ----
        with nc.allow_low_precision("bf16 exp-sum accumulation, error ~0.5% -> <1e-3 on loss"):
            for g in range(G):
                gsl = racc[:, GW * g : GW * (g + 1)]
                for m in range(T):
                    pm = ps.tile([P, GW], f32, tag="mm")
                    # k-outer: one weight load serves all 4 column slices
                    for k in range(KC):
                        for ns in range(GW // 512):
                            nc.tensor.matmul(
                                pm[:, 512 * ns : 512 * (ns + 1)],
                                imgT[k][:, P * m : P * (m + 1)],
                                specT[k][:, GW * g + 512 * ns : GW * g + 512 * (ns + 1)],
                                start=(k == 0),
                                stop=(k == KC - 1),
                            )
                    e = ep.tile([P, GW], bf16, tag="e")
                    nc.scalar.activation(
                        e, pm, FA.Exp,
                        scale=sci[:, m : m + 1],
                        accum_out=rowacc[:, m, g : g + 1],
                    )
                    if m == 0:
                        nc.vector.tensor_copy(gsl, e)
                    else:
                        nc.vector.tensor_add(out=gsl, in0=gsl, in1=e)
                # racc[g] complete: ship it out now, overlapping next g
                nc.sync.dma_start(
                    racc_o.ap()[:, GW * g : GW * (g + 1)], gsl
                )

        # ---- tails ----
        nc.vector.reduce_sum(rows, rowacc[:, :, :], axis=mybir.AxisListType.X)
        nc.sync.dma_start(rowsum_o.ap(), rows)
        nc.sync.dma_start(dotd_o.ap(), dotd)
        nc.sync.dma_start(rni_o.ap(), rni)
        nc.sync.dma_start(rns_o.ap(), rns)

    nc.compile()
    return nc


def _ensure_ntff_hook():
    """antenv.axon_hooks is absent on this image; provide the tiny get/set
    registry and register trn_agent_boot's ctypes NTFF hook so trace=True
    works. Only used from test runs (KERNEL_TRACE=1)."""
    import sys
    import types

    try:
        import antenv.axon_hooks  # noqa: F401
        return
    except ImportError:
        pass
    mod = types.ModuleType("antenv.axon_hooks")
    _state = {"hook": None}
    mod.set_axon_ntff_profile_hook = lambda h: _state.__setitem__("hook", h)
    mod.get_axon_ntff_profile_hook = lambda: _state["hook"]
    import antenv

    sys.modules["antenv.axon_hooks"] = mod
    antenv.axon_hooks = mod
    try:
        from trn_agent_boot.trn_boot import _ntff_profile_via_ctypes

        mod.set_axon_ntff_profile_hook(
            _ntff_profile_via_ctypes("/opt/axon/libaxon_pjrt.so")
        )
    except Exception as e:  # degrade to no tracing
        print(f"NTFF hook setup failed: {e}")


def kernel(image_features, spectrum_features, logit_scale):
    scale = float(np.asarray(logit_scale))
    key = round(scale, 9)
    if key not in _cache:
        _cache[key] = _build(scale)
    nc = _cache[key]

    img = np.ascontiguousarray(np.asarray(image_features, dtype=np.float32))
    spec = np.ascontiguousarray(np.asarray(spectrum_features, dtype=np.float32))
    in_maps = [
        {"img": img[c * NL : (c + 1) * NL], "spec": spec[c * NL : (c + 1) * NL]}
        for c in range(C)
    ]
    trace = os.environ.get("KERNEL_TRACE") == "1"
    if trace:
        _ensure_ntff_hook()
    res = run_bass_kernel_spmd(nc, in_maps, core_ids=list(range(C)), trace=trace)
    if trace:
        print(f"HW exec time: {res.exec_time_ns} ns (mean {res.mean_exec_time_ns})")

    rs = np.stack([r["rowsum"] for r in res.results]).astype(np.float64)   # [C,P,T]
    cs = np.stack(
        [r["racc_o"].astype(np.float64).sum(axis=0) for r in res.results]
    )  # [C,N]
    dd = np.stack([r["dotd"] for r in res.results]).astype(np.float64)
    ri = np.stack([r["rni"] for r in res.results]).astype(np.float64)
    rr = np.stack([r["rns"] for r in res.results]).astype(np.float64)

    diag_sum = float(np.sum(scale * dd * ri * rr))
    lse_i_sum = float(np.sum(np.log(rs)))
    col_total = cs.sum(axis=0)
    lse_s_sum = float(np.sum(np.log(col_total)))
    loss = 0.5 * ((lse_i_sum - diag_sum) / N + (lse_s_sum - diag_sum) / N)
    return np.float32(loss)


# revision 21
# speedup vs baseline: 1.0380x; 1.0246x over previous
"""CLIP loss kernel for trn2, 8 NeuronCores, data-parallel over the batch dim.

Strategy (per core c of 8, SPMD):
  inputs: img slice [1024, 512] f32, spec slice [1024, 512] f32 (rows
  1024c..1024c+1023 of each modality).
  1. sumsq of both slices (DVE tensor_tensor_reduce) -> 1/norms via
     exp(-0.5*ln(max(ss, eps^2))) on ACT (Ln+Exp share one table set).
  2. spec slice normalized (per-partition scalar mul, bf16 out), transposed
     via PE into [512, 1024] bf16, AllGather -> full [512, 8192] spec^T.
     img slice kept RAW (norm folded into the exp scale later), transposed
     via PE into [512, 1024] bf16.
  3. logits block: out[m=img rows, n=spec cols] = imgT.T @ specT, bf16,
     PSUM f32, tiles [128, 2048].
  4. ACT Exp with scale = logit_scale * (1/|img_row|) per partition;
     accum_out gives row-sums of exp for free. exp tile (bf16, SBUF)
     accumulated into racc[128, 8192] (DVE add) = column partial sums
     stratified by partition; final 128-partition reduce via PE ones-matmul.
  5. diag: raw img.spec dot per row (DVE), combined with norms on host.
Host: gathers per-core row-sums / column partials / diag pieces, takes logs
and means (O(N) numpy) -> scalar loss.
"""

import os
from contextlib import ExitStack

import numpy as np

import concourse.bass as bass
import concourse.mybir as mybir
from concourse import bacc, tile
from concourse.bass_utils import run_bass_kernel_spmd
from concourse.masks import make_identity

N, D, C = 8192, 512, 8
NL = N // C  # 1024 local rows per core
P = 128
T = NL // P  # 8 natural [128, 512] tiles per modality slice
KC = D // P  # 4 contraction chunks
G = 4        # column groups per core block
GW = N // G  # 2048 columns per group

f32 = mybir.dt.float32
bf16 = mybir.dt.bfloat16
FA = mybir.ActivationFunctionType
ALU = mybir.AluOpType

_cache: dict = {}


def _build(scale: float, use_cc: bool = True):
    nc = bacc.Bacc("TRN2", target_bir_lowering=False, debug=False, num_devices=C)
    img = nc.dram_tensor("img", [NL, D], f32, kind="ExternalInput")
    spec = nc.dram_tensor("spec", [NL, D], f32, kind="ExternalInput")
    rowsum_o = nc.dram_tensor("rowsum", [P, T], f32, kind="ExternalOutput")
    racc_o = nc.dram_tensor("racc_o", [P, N], bf16, kind="ExternalOutput")
    dotd_o = nc.dram_tensor("dotd", [P, T], f32, kind="ExternalOutput")
    rni_o = nc.dram_tensor("rni", [P, T], f32, kind="ExternalOutput")
    rns_o = nc.dram_tensor("rns", [P, T], f32, kind="ExternalOutput")

    with tile.TileContext(nc) as tc, ExitStack() as ctx:
        const = ctx.enter_context(tc.tile_pool(name="const", bufs=1))
        natp = ctx.enter_context(tc.tile_pool(name="nat", bufs=T))
        scp = ctx.enter_context(tc.tile_pool(name="scr", bufs=2))
        spn = ctx.enter_context(tc.tile_pool(name="specn", bufs=4))
        pers = ctx.enter_context(tc.tile_pool(name="pers", bufs=1))
        ps = ctx.enter_context(tc.tile_pool(name="ps", bufs=2, space="PSUM"))
        ep = ctx.enter_context(tc.tile_pool(name="e", bufs=3))
        dramp = ctx.enter_context(tc.tile_pool(name="dram", bufs=1, space="DRAM"))

        ident_f = const.tile([P, P], f32, name="identf")
        make_identity(nc, ident_f)
        ident_b = const.tile([P, P], bf16, name="identb")
        nc.vector.tensor_copy(ident_b, ident_f)

        imgT = [pers.tile([P, NL], bf16, name=f"imgT{k}") for k in range(KC)]
        specT = [pers.tile([P, N], bf16, name=f"specT{k}") for k in range(KC)]
        stage = [pers.tile([P, NL], bf16, name=f"stage{k}") for k in range(KC)]
        racc = pers.tile([P, N], bf16, name="racc")
        rowacc = pers.tile([P, T, G], f32, name="rowacc")
        ssi = pers.tile([P, T], f32, name="ssi")
        sss = pers.tile([P, T], f32, name="sss")
        rni = pers.tile([P, T], f32, name="rni")
        rns = pers.tile([P, T], f32, name="rns")
        sci = pers.tile([P, T], f32, name="sci")
        dotd = pers.tile([P, T], f32, name="dotd")
        lntmp = pers.tile([P, T], f32, name="lntmp")
        lntmp2 = pers.tile([P, T], f32, name="lntmp2")
        rows = pers.tile([P, T], f32, name="rows")

        # two half-column AllGathers so matmuls can start on the first half
        cc_in = [dramp.tile([D, 512], bf16, name=f"cc_in{q}") for q in range(2)]
        cc_out = [
            dramp.tile([C * D, 512], bf16, addr_space="Shared", name=f"cc_out{q}")
            for q in range(2)
        ]

        # ---- load natural tiles ----
        img_nat, spec_nat = [], []
        for t in range(T):
            st = natp.tile([P, D], f32, tag="specnat")
            nc.sync.dma_start(st, spec.ap()[t * P : (t + 1) * P, :])
            spec_nat.append(st)
        for t in range(T):
            it = natp.tile([P, D], f32, tag="imgnat")
            nc.sync.dma_start(it, img.ap()[t * P : (t + 1) * P, :])
            img_nat.append(it)

        # ---- per half: spec norms -> normalize -> transpose -> AllGather ----
        for th in range(2):
            for tt in range(4):
                t = 4 * th + tt
                s2 = scp.tile([P, D], f32, tag="scr")
                nc.scalar.activation(
                    s2, spec_nat[t], FA.Square, accum_out=sss[:, t : t + 1]
                )
            hs = slice(4 * th, 4 * th + 4)
            nc.vector.tensor_scalar_max(sss[:, hs], sss[:, hs], 1.0e-6)
            nc.scalar.activation(lntmp[:, hs], sss[:, hs], FA.Ln)
            nc.scalar.activation(rns[:, hs], lntmp[:, hs], FA.Exp, scale=-0.5)
            specn = []
            for tt in range(4):
                t = 4 * th + tt
                sn = spn.tile([P, D], bf16, tag="specn")
                nc.vector.tensor_scalar_mul(sn, spec_nat[t], rns[:, t : t + 1])
                specn.append(sn)
            for k in range(KC):
                pt = ps.tile([P, 512], bf16, tag="mm")
                for tt in range(4):
                    nc.tensor.transpose(
                        pt[:, 128 * tt : 128 * (tt + 1)],
                        specn[tt][:, 128 * k : 128 * (k + 1)],
                        ident_b,
                    )
                nc.vector.tensor_copy(stage[k][:, 512 * th : 512 * (th + 1)], pt)
                nc.sync.dma_start(
                    cc_in[th][128 * k : 128 * (k + 1), :],
                    stage[k][:, 512 * th : 512 * (th + 1)],
                )
            if use_cc:
                nc.gpsimd.collective_compute(
                    "AllGather",
                    ALU.bypass,
                    replica_groups=[list(range(C))],
                    ins=[cc_in[th].opt()],
                    outs=[cc_out[th].opt()],
                )

        # ---- img norms, diag dots, img transpose (overlap the collective) ----
        for t in range(T):
            s1 = scp.tile([P, D], f32, tag="scr")
            nc.scalar.activation(
                s1, img_nat[t], FA.Square, accum_out=ssi[:, t : t + 1]
            )
            s3 = scp.tile([P, D], f32, tag="scr")
            nc.vector.tensor_mul(out=s3, in0=img_nat[t], in1=spec_nat[t])
            nc.vector.reduce_sum(
                dotd[:, t : t + 1], s3, axis=mybir.AxisListType.X
            )
        nc.vector.tensor_scalar_max(ssi, ssi, 1.0e-6)
        nc.scalar.activation(lntmp2, ssi, FA.Ln)
        nc.scalar.activation(rni, lntmp2, FA.Exp, scale=-0.5)
        nc.vector.tensor_scalar_mul(sci, rni, scale)

        for th in range(2):
            for k in range(KC):
                pt = ps.tile([P, 512], f32, tag="mm")
                for tt in range(4):
                    t = 4 * th + tt
                    nc.tensor.transpose(
                        pt[:, 128 * tt : 128 * (tt + 1)],
                        img_nat[t][:, 128 * k : 128 * (k + 1)],
                        ident_f,
                    )
                nc.vector.tensor_copy(imgT[k][:, 512 * th : 512 * (th + 1)], pt)

        # ---- load gathered spec^T, chunk-major layout:
        # specT col 4096*q + 512*r + off  <->  global spec row 1024*r + 512*q + off
        for q in range(2):
            for r in range(C):
                for k in range(KC):
                    if use_cc:
                        src = cc_out[q][D * r + 128 * k : D * r + 128 * (k + 1), :]
                    else:  # debug: replicate the local slice (numerically wrong)
                        src = cc_in[q][128 * k : 128 * (k + 1), :]
                    nc.sync.dma_start(
                        specT[k][:, 4096 * q + 512 * r : 4096 * q + 512 * (r + 1)],
                        src,
                    )

        # ---- main loop: logits block, exp, row/col accumulation ----
        with nc.allow_low_precision("bf16 exp-sum accumulation, error ~0.5% -> <1e-3 on loss"):
            for g in range(G):
                gsl = racc[:, GW * g : GW * (g + 1)]
                for m in range(T):
                    pm = ps.tile([P, GW], f32, tag="mm")
                    # k-outer: one weight load serves all 4 column slices
                    for k in range(KC):
                        for ns in range(GW // 512):
                            nc.tensor.matmul(
                                pm[:, 512 * ns : 512 * (ns + 1)],
                                imgT[k][:, P * m : P * (m + 1)],
                                specT[k][:, GW * g + 512 * ns : GW * g + 512 * (ns + 1)],
                                start=(k == 0),
                                stop=(k == KC - 1),
                            )
                    e = ep.tile([P, GW], bf16, tag="e")
                    nc.scalar.activation(
                        e, pm, FA.Exp,
                        scale=sci[:, m : m + 1],
                        accum_out=rowacc[:, m, g : g + 1],
                    )
                    if m == 0:
                        nc.vector.tensor_copy(gsl, e)
                    else:
                        nc.vector.tensor_add(out=gsl, in0=gsl, in1=e)
                # racc[g] complete: ship it out now, overlapping next g
                nc.sync.dma_start(
                    racc_o.ap()[:, GW * g : GW * (g + 1)], gsl
                )

        # ---- tails ----
        nc.vector.reduce_sum(rows, rowacc[:, :, :], axis=mybir.AxisListType.X)
        nc.sync.dma_start(rowsum_o.ap(), rows)
        nc.sync.dma_start(dotd_o.ap(), dotd)
        nc.sync.dma_start(rni_o.ap(), rni)
        nc.sync.dma_start(rns_o.ap(), rns)

    nc.compile()
    return nc


def _ensure_ntff_hook():
    """antenv.axon_hooks is absent on this image; provide the tiny get/set
    registry and register trn_agent_boot's ctypes NTFF hook so trace=True
    works. Only used from test runs (KERNEL_TRACE=1)."""
    import sys
    import types

    try:
        import antenv.axon_hooks  # noqa: F401
        return
    except ImportError:
        pass
    mod = types.ModuleType("antenv.axon_hooks")
    _state = {"hook": None}
    mod.set_axon_ntff_profile_hook = lambda h: _state.__setitem__("hook", h)
    mod.get_axon_ntff_profile_hook = lambda: _state["hook"]
    import antenv

    sys.modules["antenv.axon_hooks"] = mod
    antenv.axon_hooks = mod
    try:
        from trn_agent_boot.trn_boot import _ntff_profile_via_ctypes

        mod.set_axon_ntff_profile_hook(
            _ntff_profile_via_ctypes("/opt/axon/libaxon_pjrt.so")
        )
    except Exception as e:  # degrade to no tracing
        print(f"NTFF hook setup failed: {e}")


def kernel(image_features, spectrum_features, logit_scale):
    scale = float(np.asarray(logit_scale))
    key = round(scale, 9)
    if key not in _cache:
        _cache[key] = _build(scale)
    nc = _cache[key]

    img = np.ascontiguousarray(np.asarray(image_features, dtype=np.float32))
    spec = np.ascontiguousarray(np.asarray(spectrum_features, dtype=np.float32))
    in_maps = [
        {"img": img[c * NL : (c + 1) * NL], "spec": spec[c * NL : (c + 1) * NL]}
        for c in range(C)
    ]
    trace = os.environ.get("KERNEL_TRACE") == "1"
    if trace:
        _ensure_ntff_hook()
    res = run_bass_kernel_spmd(nc, in_maps, core_ids=list(range(C)), trace=trace)
    if trace:
        print(f"HW exec time: {res.exec_time_ns} ns (mean {res.mean_exec_time_ns})")

    rs = np.stack([r["rowsum"] for r in res.results]).astype(np.float64)   # [C,P,T]
    cs = np.stack(
        [r["racc_o"].astype(np.float64).sum(axis=0) for r in res.results]
    )  # [C,N]
    dd = np.stack([r["dotd"] for r in res.results]).astype(np.float64)
    ri = np.stack([r["rni"] for r in res.results]).astype(np.float64)
    rr = np.stack([r["rns"] for r in res.results]).astype(np.float64)

    diag_sum = float(np.sum(scale * dd * ri * rr))
    lse_i_sum = float(np.sum(np.log(rs)))
    col_total = cs.sum(axis=0)  # still in device (chunk-major) column order
    lse_s_sum = float(np.sum(np.log(col_total)))
    loss = 0.5 * ((lse_i_sum - diag_sum) / N + (lse_s_sum - diag_sum) / N)
    return np.float32(loss)


# revision 30
# speedup vs baseline: 1.4829x; 1.4286x over previous
"""CLIP loss kernel for trn2, 8 NeuronCores, data-parallel over the batch dim.

Strategy (per core c of 8, SPMD):
  inputs: img slice [1024, 512] f32, spec slice [1024, 512] f32 (rows
  1024c..1024c+1023 of each modality).
  1. sumsq of both slices (DVE tensor_tensor_reduce) -> 1/norms via
     exp(-0.5*ln(max(ss, eps^2))) on ACT (Ln+Exp share one table set).
  2. spec slice normalized (per-partition scalar mul, bf16 out), transposed
     via PE into [512, 1024] bf16, AllGather -> full [512, 8192] spec^T.
     img slice kept RAW (norm folded into the exp scale later), transposed
     via PE into [512, 1024] bf16.
  3. logits block: out[m=img rows, n=spec cols] = imgT.T @ specT, bf16,
     PSUM f32, tiles [128, 2048].
  4. ACT Exp with scale = logit_scale * (1/|img_row|) per partition;
     accum_out gives row-sums of exp for free. exp tile (bf16, SBUF)
     accumulated into racc[128, 8192] (DVE add) = column partial sums
     stratified by partition; final 128-partition reduce via PE ones-matmul.
  5. diag: raw img.spec dot per row (DVE), combined with norms on host.
Host: gathers per-core row-sums / column partials / diag pieces, takes logs
and means (O(N) numpy) -> scalar loss.
"""

import os
from contextlib import ExitStack

import numpy as np

import concourse.bass as bass
import concourse.mybir as mybir
from concourse import bacc, tile
from concourse.bass_utils import run_bass_kernel_spmd
from concourse.masks import make_identity

N, D, C = 8192, 512, 8
NL = N // C  # 1024 local rows per core
P = 128
T = NL // P  # 8 natural [128, 512] tiles per modality slice
KC = D // P  # 4 contraction chunks
G = 4        # column groups per core block
GW = N // G  # 2048 columns per group

f32 = mybir.dt.float32
bf16 = mybir.dt.bfloat16
fp8 = mybir.dt.float8e4
FA = mybir.ActivationFunctionType
ALU = mybir.AluOpType

# fp8 operands are pre-scaled by 16 to stay out of the subnormal range;
# the matmul result is 16x too big on the spec side only (img kept raw),
# compensated in the exp scale.
FP8_PRESCALE = 16.0

_cache: dict = {}


def _build(scale: float, use_cc: bool = True):
    nc = bacc.Bacc("TRN2", target_bir_lowering=False, debug=False, num_devices=C)
    img = nc.dram_tensor("img", [NL, D], f32, kind="ExternalInput")
    spec = nc.dram_tensor("spec", [NL, D], f32, kind="ExternalInput")
    rowsum_o = nc.dram_tensor("rowsum", [P, T], f32, kind="ExternalOutput")
    racc_o = nc.dram_tensor("racc_o", [P, N], bf16, kind="ExternalOutput")
    dotd_o = nc.dram_tensor("dotd", [P, T], f32, kind="ExternalOutput")
    rni_o = nc.dram_tensor("rni", [P, T], f32, kind="ExternalOutput")
    rns_o = nc.dram_tensor("rns", [P, T], f32, kind="ExternalOutput")

    with tile.TileContext(nc) as tc, ExitStack() as ctx:
        const = ctx.enter_context(tc.tile_pool(name="const", bufs=1))
        natp = ctx.enter_context(tc.tile_pool(name="nat", bufs=T))
        scp = ctx.enter_context(tc.tile_pool(name="scr", bufs=2))
        spn = ctx.enter_context(tc.tile_pool(name="specn", bufs=4))
        pers = ctx.enter_context(tc.tile_pool(name="pers", bufs=1))
        ps = ctx.enter_context(tc.tile_pool(name="ps", bufs=2, space="PSUM"))
        ep = ctx.enter_context(tc.tile_pool(name="e", bufs=3))
        dramp = ctx.enter_context(tc.tile_pool(name="dram", bufs=1, space="DRAM"))

        ident_f = const.tile([P, P], f32, name="identf")
        make_identity(nc, ident_f)
        ident_b = const.tile([P, P], bf16, name="identb")
        nc.vector.tensor_copy(ident_b, ident_f)

        imgT = pers.tile([P, KC, NL], fp8, name="imgT")
        specT = pers.tile([P, KC, N], fp8, name="specT")
        stage = [pers.tile([P, NL], fp8, name=f"stage{k}") for k in range(KC)]
        racc = pers.tile([P, N], bf16, name="racc")
        rowacc = pers.tile([P, T, G], f32, name="rowacc")
        ssi = pers.tile([P, T], f32, name="ssi")
        sss = pers.tile([P, T], f32, name="sss")
        rni = pers.tile([P, T], f32, name="rni")
        rns = pers.tile([P, T], f32, name="rns")
        sci = pers.tile([P, T], f32, name="sci")
        rns16 = pers.tile([P, T], f32, name="rns16")
        dotd = pers.tile([P, T], f32, name="dotd")
        lntmp = pers.tile([P, T], f32, name="lntmp")
        lntmp2 = pers.tile([P, T], f32, name="lntmp2")
        rows = pers.tile([P, T], f32, name="rows")

        # two half-column AllGathers so matmuls can start on the first half
        cc_in = [dramp.tile([D, 512], fp8, name=f"cc_in{q}") for q in range(2)]
        cc_out = [
            dramp.tile([C * D, 512], fp8, addr_space="Shared", name=f"cc_out{q}")
            for q in range(2)
        ]

        # ---- load natural tiles ----
        img_nat, spec_nat = [], []
        for t in range(T):
            st = natp.tile([P, D], f32, tag="specnat")
            nc.sync.dma_start(st, spec.ap()[t * P : (t + 1) * P, :])
            spec_nat.append(st)
        for t in range(T):
            it = natp.tile([P, D], f32, tag="imgnat")
            nc.sync.dma_start(it, img.ap()[t * P : (t + 1) * P, :])
            img_nat.append(it)

        # ---- per half: spec norms -> normalize -> transpose -> AllGather ----
        for th in range(2):
            for tt in range(4):
                t = 4 * th + tt
                s2 = scp.tile([P, D], f32, tag="scr")
                nc.scalar.activation(
                    s2, spec_nat[t], FA.Square, accum_out=sss[:, t : t + 1]
                )
            hs = slice(4 * th, 4 * th + 4)
            nc.vector.tensor_scalar_max(sss[:, hs], sss[:, hs], 1.0e-6)
            nc.scalar.activation(lntmp[:, hs], sss[:, hs], FA.Ln)
            nc.scalar.activation(rns[:, hs], lntmp[:, hs], FA.Exp, scale=-0.5)
            nc.vector.tensor_scalar_mul(rns16[:, hs], rns[:, hs], FP8_PRESCALE)
            specn = []
            for tt in range(4):
                t = 4 * th + tt
                sn = spn.tile([P, D], bf16, tag="specn")
                nc.vector.tensor_scalar_mul(sn, spec_nat[t], rns16[:, t : t + 1])
                specn.append(sn)
            for k in range(KC):
                pt = ps.tile([P, 512], bf16, tag="mm")
                for tt in range(4):
                    nc.tensor.transpose(
                        pt[:, 128 * tt : 128 * (tt + 1)],
                        specn[tt][:, 128 * k : 128 * (k + 1)],
                        ident_b,
                    )
                nc.vector.tensor_copy(stage[k][:, 512 * th : 512 * (th + 1)], pt)
                nc.sync.dma_start(
                    cc_in[th][128 * k : 128 * (k + 1), :],
                    stage[k][:, 512 * th : 512 * (th + 1)],
                )
            if use_cc:
                nc.gpsimd.collective_compute(
                    "AllGather",
                    ALU.bypass,
                    replica_groups=[list(range(C))],
                    ins=[cc_in[th].opt()],
                    outs=[cc_out[th].opt()],
                )

        # ---- img norms, diag dots, img transpose (overlap the collective) ----
        for t in range(T):
            s1 = scp.tile([P, D], f32, tag="scr")
            nc.scalar.activation(
                s1, img_nat[t], FA.Square, accum_out=ssi[:, t : t + 1]
            )
            s3 = scp.tile([P, D], f32, tag="scr")
            nc.vector.tensor_mul(out=s3, in0=img_nat[t], in1=spec_nat[t])
            nc.vector.reduce_sum(
                dotd[:, t : t + 1], s3, axis=mybir.AxisListType.X
            )
        nc.vector.tensor_scalar_max(ssi, ssi, 1.0e-6)
        nc.scalar.activation(lntmp2, ssi, FA.Ln)
        nc.scalar.activation(rni, lntmp2, FA.Exp, scale=-0.5)
        nc.vector.tensor_scalar_mul(sci, rni, scale / FP8_PRESCALE)

        for th in range(2):
            for k in range(KC):
                pt = ps.tile([P, 512], f32, tag="mm")
                for tt in range(4):
                    t = 4 * th + tt
                    nc.tensor.transpose(
                        pt[:, 128 * tt : 128 * (tt + 1)],
                        img_nat[t][:, 128 * k : 128 * (k + 1)],
                        ident_f,
                    )
                nc.vector.tensor_copy(imgT[:, k, 512 * th : 512 * (th + 1)], pt)

        # ---- load gathered spec^T, chunk-major layout:
        # specT col 4096*q + 512*r + off  <->  global spec row 1024*r + 512*q + off
        for q in range(2):
            for r in range(C):
                for k in range(KC):
                    if use_cc:
                        src = cc_out[q][D * r + 128 * k : D * r + 128 * (k + 1), :]
                    else:  # debug: replicate the local slice (numerically wrong)
                        src = cc_in[q][128 * k : 128 * (k + 1), :]
                    nc.sync.dma_start(
                        specT[:, k, 4096 * q + 512 * r : 4096 * q + 512 * (r + 1)],
                        src,
                    )

        # ---- main loop: logits block, exp, row/col accumulation ----
        with nc.allow_low_precision("bf16 exp-sum accumulation, error ~0.5% -> <1e-3 on loss"):
            for g in range(G):
                gsl = racc[:, GW * g : GW * (g + 1)]
                for m in range(T):
                    pm = ps.tile([P, GW], f32, tag="mm")
                    # fp8 DoubleRow: each matmul contracts 2 k-chunks (K=256)
                    for q in range(KC // 2):
                        for ns in range(GW // 512):
                            cs = slice(GW * g + 512 * ns, GW * g + 512 * (ns + 1))
                            nc.tensor.matmul(
                                pm[:, 512 * ns : 512 * (ns + 1)],
                                imgT[:, 2 * q : 2 * q + 2, P * m : P * (m + 1)],
                                specT[:, 2 * q : 2 * q + 2, cs],
                                start=(q == 0),
                                stop=(q == KC // 2 - 1),
                                perf_mode=mybir.MatmulPerfMode.DoubleRow,
                            )
                    e = ep.tile([P, GW], bf16, tag="e")
                    nc.scalar.activation(
                        e, pm, FA.Exp,
                        scale=sci[:, m : m + 1],
                        accum_out=rowacc[:, m, g : g + 1],
                    )
                    if m == 0:
                        nc.vector.tensor_copy(gsl, e)
                    else:
                        nc.vector.tensor_add(out=gsl, in0=gsl, in1=e)
                # racc[g] complete: ship it out now, overlapping next g
                nc.sync.dma_start(
                    racc_o.ap()[:, GW * g : GW * (g + 1)], gsl
                )

        # ---- tails ----
        nc.vector.reduce_sum(rows, rowacc[:, :, :], axis=mybir.AxisListType.X)
        nc.sync.dma_start(rowsum_o.ap(), rows)
        nc.sync.dma_start(dotd_o.ap(), dotd)
        nc.sync.dma_start(rni_o.ap(), rni)
        nc.sync.dma_start(rns_o.ap(), rns)

    nc.compile()
    return nc


def _ensure_ntff_hook():
    """antenv.axon_hooks is absent on this image; provide the tiny get/set
    registry and register trn_agent_boot's ctypes NTFF hook so trace=True
    works. Only used from test runs (KERNEL_TRACE=1)."""
    import sys
    import types

    try:
        import antenv.axon_hooks  # noqa: F401
        return
    except ImportError:
        pass
    mod = types.ModuleType("antenv.axon_hooks")
    _state = {"hook": None}
    mod.set_axon_ntff_profile_hook = lambda h: _state.__setitem__("hook", h)
    mod.get_axon_ntff_profile_hook = lambda: _state["hook"]
    import antenv

    sys.modules["antenv.axon_hooks"] = mod
    antenv.axon_hooks = mod
    try:
        from trn_agent_boot.trn_boot import _ntff_profile_via_ctypes

        mod.set_axon_ntff_profile_hook(
            _ntff_profile_via_ctypes("/opt/axon/libaxon_pjrt.so")
        )
    except Exception as e:  # degrade to no tracing
        print(f"NTFF hook setup failed: {e}")


def kernel(image_features, spectrum_features, logit_scale):
    scale = float(np.asarray(logit_scale))
    key = round(scale, 9)
    if key not in _cache:
        _cache[key] = _build(scale)
    nc = _cache[key]

    img = np.ascontiguousarray(np.asarray(image_features, dtype=np.float32))
    spec = np.ascontiguousarray(np.asarray(spectrum_features, dtype=np.float32))
    in_maps = [
        {"img": img[c * NL : (c + 1) * NL], "spec": spec[c * NL : (c + 1) * NL]}
        for c in range(C)
    ]
    trace = os.environ.get("KERNEL_TRACE") == "1"
    if trace:
        _ensure_ntff_hook()
    res = run_bass_kernel_spmd(nc, in_maps, core_ids=list(range(C)), trace=trace)
    if trace:
        print(f"HW exec time: {res.exec_time_ns} ns (mean {res.mean_exec_time_ns})")

    rs = np.stack([r["rowsum"] for r in res.results]).astype(np.float64)   # [C,P,T]
    cs = np.stack(
        [r["racc_o"].astype(np.float64).sum(axis=0) for r in res.results]
    )  # [C,N]
    dd = np.stack([r["dotd"] for r in res.results]).astype(np.float64)
    ri = np.stack([r["rni"] for r in res.results]).astype(np.float64)
    rr = np.stack([r["rns"] for r in res.results]).astype(np.float64)

    diag_sum = float(np.sum(scale * dd * ri * rr))
    lse_i_sum = float(np.sum(np.log(rs)))
    col_total = cs.sum(axis=0)  # still in device (chunk-major) column order
    lse_s_sum = float(np.sum(np.log(col_total)))
    loss = 0.5 * ((lse_i_sum - diag_sum) / N + (lse_s_sum - diag_sum) / N)
    return np.float32(loss)
